# revision 13
# baseline (speedup 1.0000x reference)
"""Trainium2 Bass kernel for nn_DecoderInitWrapper (4-layer decoder prefill).

Strategy: sequence-parallel across 8 NeuronCores (256 rows each), weights
replicated (fp16), per-layer AllGather of K^T and V' (V with a fused ones
column so the softmax denominator falls out of the PV matmul). Attention is
computed in S^T layout (keys on partitions): softmax needs no max-subtraction
because q/k are RMS-normalized so |s| <= 8; causality is a multiplicative
per-core {0,1} mask applied after exp. All matmuls fp16 with fp32 PSUM
accumulation; residual stream, norms, rope, softmax bookkeeping in fp32.

Host side: embedding gather + audio scatter; RoPE tables with qn/kn and the
1/sqrt(hd) scaling folded in; ln1/ln2/final-norm weights folded into the
following projection matrices.

Self-contained: hardcodes all shapes; reads no files.
"""
import sys

for _p in ("/opt/trn_rl_repo",):
    if _p not in sys.path:
        sys.path.insert(0, _p)

import contextlib

import numpy as np

import concourse.mybir as mybir
import concourse.bacc as bacc
import concourse.tile as tile
from concourse import bass_utils

F32 = mybir.dt.float32
F16 = mybir.dt.float16

B, S, V, H = 1, 2048, 32000, 1024
L, NH, NKV, HD, I = 4, 16, 4, 64, 4096
A = 256
THETA = 10000.0
EPS = 1e-6
SCALING = HD ** -0.5
GQ = NH // NKV
N_CORES = 8

KVW = NKV * HD          # 256

# Head->slot permutation: qT/attnT store head h in slot SLOT[h]; slots with
# even index sit at partition 0, odd at partition 64. Chosen so slot parity
# equals kv-group parity (g = h // GQ), matching kT's partition offset in the
# allgather layout. Host permutes qw columns / ow rows to this order.
HEAD_AT_SLOT = []
for _i in range(NH // 2):
    HEAD_AT_SLOT.append((_i // GQ) * 2 * GQ + (_i % GQ))          # even-g heads
    HEAD_AT_SLOT.append((_i // GQ) * 2 * GQ + GQ + (_i % GQ))     # odd-g heads
SLOT = [0] * NH
for _s, _h in enumerate(HEAD_AT_SLOT):
    SLOT[_h] = _s

VW1 = NKV * (HD + 1)    # 260
AGW = KVW + VW1         # 516


class Cfg:
    def __init__(self, n_cores=N_CORES, s=S, l=L, v=V, i_dim=I):
        self.NC, self.S, self.L, self.V, self.I = n_cores, s, l, v, i_dim
        self.R = s // n_cores
        self.NRT = self.R // 128
        self.NKT = s // 128
        self.FC = H // 128
        self.IC = i_dim // 128
        self.NVC = (v + 511) // 512
        self.VC_LAST = v - 512 * (v // 512) or 512
        assert self.R % 128 == 0


def build_nc(cfg: Cfg):
    nc = bacc.Bacc("TRN2", target_bir_lowering=False, debug=False,
                   num_devices=cfg.NC)
    d = {}
    R, Ldim, Vdim, Idim = cfg.R, cfg.L, cfg.V, cfg.I

    def inp(name, shape, dt=F16):
        d[name] = nc.dram_tensor(name, shape, dt, kind="ExternalInput").ap()

    inp("h0", [R, H], F32)
    inp("cosq", [Ldim, R, NH * HD], F32)
    inp("sinq", [Ldim, R, NH * HD], F32)
    inp("cosk", [Ldim, R, KVW], F32)
    inp("sink", [Ldim, R, KVW], F32)
    inp("mask", [cfg.S, R], F16)
    inp("ident", [128, 128], F16)
    inp("qw", [Ldim, H, NH * HD])
    inp("kw", [Ldim, H, KVW])
    inp("vw", [Ldim, H, KVW])
    inp("ow", [Ldim, NH * HD, H])
    inp("gw", [Ldim, H, Idim])
    inp("uw", [Ldim, H, Idim])
    inp("dw", [Ldim, Idim, H])
    inp("lm", [H, Vdim])

    logits = nc.dram_tensor("logits", [R, Vdim], F32, kind="ExternalOutput").ap()
    pk = nc.dram_tensor("pk", [Ldim, NKV, R, HD], F32, kind="ExternalOutput").ap()
    pv = nc.dram_tensor("pv", [Ldim, NKV, R, HD], F32, kind="ExternalOutput").ap()

    with tile.TileContext(nc) as tc:
        _emit(nc, tc, cfg, d, logits, pk, pv)
    nc.compile()
    return nc


def _emit(nc, tc, cfg, d, logits, pk, pv):
    R, NRT, NKT, FC, IC = cfg.R, cfg.NRT, cfg.NKT, cfg.FC, cfg.IC
    Ldim = cfg.L
    NJ = H // 512  # 512-chunks of H

    ctx = contextlib.ExitStack()
    with ctx:
        cpool = ctx.enter_context(tc.tile_pool(name="const", bufs=1))
        ident = cpool.tile([128, 128], F16, name="ident")
        nc.sync.dma_start(ident[:], d["ident"][:])
        one1 = cpool.tile([1, 128], F32, name="one1")
        nc.gpsimd.memset(one1[:], 1.0)
        epst = cpool.tile([128, 1], F32, name="epst")
        nc.gpsimd.memset(epst[:], EPS)
        maskt = cpool.tile([128, NKT, 2 * R], F16, name="maskt")
        nc.sync.dma_start(maskt[:, :, 0:R],
                          d["mask"].rearrange("(kt p) r -> p kt r", p=128))
        nc.sync.dma_start(maskt[:, :, R:2 * R],
                          d["mask"].rearrange("(kt p) r -> p kt r", p=128))
        h = cpool.tile([128, NRT, H], F32, name="h")
        nc.sync.dma_start(h[:], d["h0"].rearrange("(rt p) x -> p rt x", p=128))

        apool = ctx.enter_context(tc.tile_pool(name="acts", bufs=1))
        rpool = ctx.enter_context(tc.tile_pool(name="ropetabs", bufs=2))
        dpool = ctx.enter_context(tc.tile_pool(name="dramp", bufs=2, space="DRAM"))

        def rmsnorm_T(tag):
            """normedT [128, FC, R] f16 = transpose(h * rsqrt(mean(h^2)+eps))."""
            normedT = apool.tile([128, FC, R], F16, name="normedT", tag="normedT")
            with (
                tc.tile_pool(name=f"rms_{tag}", bufs=2) as rp,
                tc.tile_pool(name=f"rmsps_{tag}", bufs=2, space="PSUM") as pp,
            ):
                for rt in range(NRT):
                    sq = rp.tile([128, H], F16, name="sq", tag="sq")
                    var = rp.tile([128, 1], F32, name="var", tag="var")
                    nc.scalar.activation(sq[:], h[:, rt, :],
                                         mybir.ActivationFunctionType.Square,
                                         scale=float(1.0 / np.sqrt(H)),
                                         accum_out=var[:])
                    std = rp.tile([128, 1], F32, name="std", tag="std")
                    nc.scalar.activation(std[:], var[:],
                                         mybir.ActivationFunctionType.Sqrt,
                                         bias=epst[:])
                    rs = rp.tile([128, 1], F32, name="rs", tag="rs")
                    nc.vector.reciprocal(rs[:], std[:])
                    normed = rp.tile([128, H], F16, name="normed", tag="normed")
                    nc.vector.tensor_scalar_mul(normed[:], h[:, rt, :], rs[:])
                    for fc in range(FC):
                        tp = pp.tile([128, 128], F16, name="tp", tag="tp")
                        nc.tensor.transpose(tp[:], normed[:, fc * 128:(fc + 1) * 128],
                                            ident[:])
                        nc.vector.tensor_copy(
                            normedT[:, fc, rt * 128:(rt + 1) * 128], tp[:])
            return normedT

        def rope_inplace(dst_ap, xn, cos_ap, sin_ap, nheads, f32_copy=None):
            """dst = xn*cos + rotview(xn)*sin; all [128, nheads*64] fp32 views."""
            with tc.tile_pool(name="ropetmp", bufs=2) as rp:
                W = nheads * HD
                t1 = rp.tile([128, W], F32, name="t1", tag=f"t1{nheads}")
                nc.vector.tensor_tensor(out=t1[:], in0=xn[:], in1=cos_ap,
                                        op=mybir.AluOpType.mult)
                t2 = rp.tile([128, W], F32, name="t2", tag=f"t2{nheads}")
                xv = xn[:].rearrange("p (h b x) -> p h b x", h=nheads, b=2, x=32)
                sv = sin_ap.rearrange("p (h b x) -> p h b x", h=nheads, b=2, x=32)
                tv = t2[:].rearrange("p (h b x) -> p h b x", h=nheads, b=2, x=32)
                # block-swapped multiply (rotate-half with sign folded into sin)
                nc.vector.tensor_tensor(out=tv[:, :, 0, :], in0=xv[:, :, 1, :],
                                        in1=sv[:, :, 0, :], op=mybir.AluOpType.mult)
                nc.vector.tensor_tensor(out=tv[:, :, 1, :], in0=xv[:, :, 0, :],
                                        in1=sv[:, :, 1, :], op=mybir.AluOpType.mult)
                nc.vector.tensor_tensor(out=dst_ap, in0=t1[:], in1=t2[:],
                                        op=mybir.AluOpType.add)
                if f32_copy is not None:
                    nc.vector.tensor_tensor(out=f32_copy, in0=t1[:], in1=t2[:],
                                            op=mybir.AluOpType.add)

        for l in range(Ldim):
            # rope tables for this layer
            cosq = rpool.tile([128, NRT, NH * HD], F32, name="cosq", tag="cosq")
            sinq = rpool.tile([128, NRT, NH * HD], F32, name="sinq", tag="sinq")
            cosk = rpool.tile([128, NRT, KVW], F32, name="cosk", tag="cosk")
            sink = rpool.tile([128, NRT, KVW], F32, name="sink", tag="sink")
            for nm, t in (("cosq", cosq), ("sinq", sinq),
                          ("cosk", cosk), ("sink", sink)):
                nc.sync.dma_start(t[:],
                                  d[nm][l].rearrange("(rt p) x -> p rt x", p=128))

            # ---------------- ln1 + qkv + rope ----------------
            _sc1 = nc.named_scope(f"ln1_{l}"); _sc1.__enter__()
            normedT = rmsnorm_T(f"l{l}a")
            _sc1.__exit__(None, None, None)
            _sc = nc.named_scope(f"qkv_{l}"); _sc.__enter__()
            q16 = apool.tile([128, NRT, NH * HD], F16, name="q16", tag="q16")
            k16 = apool.tile([128, NRT, KVW], F16, name="k16", tag="k16")
            kf32 = apool.tile([128, NRT, KVW], F32, name="kf32", tag="kf32")
            vf = apool.tile([128, NRT, VW1], F16, name="vf", tag="vf")
            nc.gpsimd.memset(
                vf[:].rearrange("p rt (g x) -> p rt g x", g=NKV, x=HD + 1)
                [:, :, :, HD:HD + 1], 1.0)

            with (
                tc.tile_pool(name=f"qkvw_{l}", bufs=3) as wpool,
                tc.tile_pool(name=f"qkvt_{l}", bufs=2) as tpool,
                tc.tile_pool(name=f"qkvp_{l}", bufs=2, space="PSUM") as pq,
            ):
                for rt in range(NRT):
                    psq = [pq.tile([128, 512], F32, name=f"psq{j}", tag=f"psq{j}")
                           for j in range(NH * HD // 512)]
                    psk = pq.tile([128, KVW], F32, name="psk", tag="psk")
                    psv = pq.tile([128, KVW], F32, name="psv", tag="psv")
                    for fc in range(FC):
                        wsl = wpool.tile([128, NH * HD + 2 * KVW], F16,
                                         name="wsl", tag="wsl")
                        nc.sync.dma_start(wsl[:, 0:NH * HD],
                                          d["qw"][l, fc * 128:(fc + 1) * 128, :])
                        nc.sync.dma_start(wsl[:, NH * HD:NH * HD + KVW],
                                          d["kw"][l, fc * 128:(fc + 1) * 128, :])
                        nc.sync.dma_start(wsl[:, NH * HD + KVW:],
                                          d["vw"][l, fc * 128:(fc + 1) * 128, :])
                        lhsT = normedT[:, fc, rt * 128:(rt + 1) * 128]
                        st = dict(start=(fc == 0), stop=(fc == FC - 1))
                        for j in range(NH * HD // 512):
                            nc.tensor.matmul(psq[j][:], lhsT,
                                             wsl[:, j * 512:(j + 1) * 512], **st)
                        nc.tensor.matmul(psk[:], lhsT,
                                         wsl[:, NH * HD:NH * HD + KVW], **st)
                        nc.tensor.matmul(psv[:], lhsT,
                                         wsl[:, NH * HD + KVW:], **st)

                    # ---- q epilogue: per-head rms + rope -> q16
                    xnq = tpool.tile([128, NH * HD], F32, name="xnq", tag="xnq")
                    ssq = tpool.tile([128, NH], F32, name="ssq", tag="ssq")
                    scr = tpool.tile([128, HD], F16, name="scr", tag="scr")
                    for hh in range(NH):
                        bank = psq[(hh * HD) // 512]
                        off = (hh * HD) % 512
                        nc.scalar.activation(scr[:], bank[:, off:off + HD],
                                             mybir.ActivationFunctionType.Square,
                                             scale=float(1.0 / np.sqrt(HD)),
                                             accum_out=ssq[:, hh:hh + 1])
                    stq = tpool.tile([128, NH], F32, name="stq", tag="stq")
                    nc.scalar.activation(stq[:], ssq[:],
                                         mybir.ActivationFunctionType.Sqrt,
                                         bias=epst[:])
                    rsq = tpool.tile([128, NH], F32, name="rsq", tag="rsq")
                    nc.vector.reciprocal(rsq[:], stq[:])
                    for hh in range(NH):
                        bank = psq[(hh * HD) // 512]
                        off = (hh * HD) % 512
                        nc.vector.tensor_scalar_mul(
                            xnq[:, hh * HD:(hh + 1) * HD],
                            bank[:, off:off + HD], rsq[:, hh:hh + 1])
                    rope_inplace(q16[:, rt, :], xnq, cosq[:, rt, :],
                                 sinq[:, rt, :], NH)

                    # ---- k epilogue
                    xnk = tpool.tile([128, KVW], F32, name="xnk", tag="xnk")
                    ssk = tpool.tile([128, NKV], F32, name="ssk", tag="ssk")
                    for hh in range(NKV):
                        nc.scalar.activation(scr[:], psk[:, hh * HD:(hh + 1) * HD],
                                             mybir.ActivationFunctionType.Square,
                                             scale=float(1.0 / np.sqrt(HD)),
                                             accum_out=ssk[:, hh:hh + 1])
                    stk = tpool.tile([128, NKV], F32, name="stk", tag="stk")
                    nc.scalar.activation(stk[:], ssk[:],
                                         mybir.ActivationFunctionType.Sqrt,
                                         bias=epst[:])
                    rsk = tpool.tile([128, NKV], F32, name="rsk", tag="rsk")
                    nc.vector.reciprocal(rsk[:], stk[:])
                    for hh in range(NKV):
                        nc.vector.tensor_scalar_mul(
                            xnk[:, hh * HD:(hh + 1) * HD],
                            psk[:, hh * HD:(hh + 1) * HD], rsk[:, hh:hh + 1])
                    rope_inplace(k16[:, rt, :], xnk, cosk[:, rt, :],
                                 sink[:, rt, :], NKV, f32_copy=kf32[:, rt, :])

                    # ---- v epilogue
                    vf32 = tpool.tile([128, KVW], F32, name="vf32", tag="vf32")
                    nc.scalar.copy(vf32[:], psv[:])
                    nc.sync.dma_start(
                        pv[l].rearrange("g (rt p) x -> p rt g x", p=128)[:, rt],
                        vf32[:].rearrange("p (g x) -> p g x", g=NKV))
                    nc.vector.tensor_copy(
                        vf[:, rt, :].rearrange("p (g x) -> p g x", g=NKV, x=HD + 1)
                        [:, :, 0:HD],
                        psv[:].rearrange("p (g x) -> p g x", g=NKV))
                for rt in range(NRT):
                    nc.sync.dma_start(
                        pk[l].rearrange("g (rt p) x -> p rt g x", p=128)[:, rt],
                        kf32[:, rt, :].rearrange("p (g x) -> p g x", g=NKV))

            _sc.__exit__(None, None, None)
            _sc = nc.named_scope(f"tps_{l}"); _sc.__enter__()
            # ---------------- transposes: qT, kT ----------------
            qT = apool.tile([128, FC, R], F16, name="qT", tag="qT")
            kT = apool.tile([128, KVW // 128, R], F16, name="kT", tag="kT")
            with tc.tile_pool(name=f"tps_{l}", bufs=2, space="PSUM") as pp:
                for rt in range(NRT):
                    for fc in range(FC):
                        tp = pp.tile([128, 128], F16, name="tp", tag="tp")
                        nc.tensor.transpose(
                            tp[:], q16[:, rt, fc * 128:(fc + 1) * 128], ident[:])
                        nc.vector.tensor_copy(
                            qT[:, fc, rt * 128:(rt + 1) * 128], tp[:])
                    for cb in range(KVW // 128):
                        tp = pp.tile([128, 128], F16, name="tp", tag="tp")
                        nc.tensor.transpose(
                            tp[:], k16[:, rt, cb * 128:(cb + 1) * 128], ident[:])
                        nc.vector.tensor_copy(
                            kT[:, cb, rt * 128:(rt + 1) * 128], tp[:])

            _sc.__exit__(None, None, None)
            _sc = nc.named_scope(f"ag_{l}"); _sc.__enter__()
            # ---------------- AllGather kT and v' ----------------
            shared = "Shared" if cfg.NC > 4 else "Local"
            kagin = dpool.tile([KVW, R], F16, name="kagin", tag="kagin")
            vagin = dpool.tile([R, VW1], F16, name="vagin", tag="vagin")
            kagout = dpool.tile([cfg.NC * KVW, R], F16, name="kagout",
                                tag="kagout", addr_space=shared)
            vagout = dpool.tile([cfg.NC * R, VW1], F16, name="vagout",
                                tag="vagout", addr_space=shared)
            nc.sync.dma_start(
                kagin[:].rearrange("(cb p) r -> p cb r", p=128), kT[:])
            nc.sync.dma_start(
                vagin[:].rearrange("(rt p) x -> p rt x", p=128), vf[:])
            nc.gpsimd.collective_compute(
                "AllGather", mybir.AluOpType.bypass,
                replica_groups=[list(range(cfg.NC))],
                ins=[kagin.opt()], outs=[kagout.opt()])
            nc.gpsimd.collective_compute(
                "AllGather", mybir.AluOpType.bypass,
                replica_groups=[list(range(cfg.NC))],
                ins=[vagin.opt()], outs=[vagout.opt()])
            NKB = KVW // 128  # kT partition blocks per core
            ktag = apool.tile([128, cfg.NC * NKB, R], F16, name="ktag",
                              tag="ktag")
            vag = apool.tile([128, NKT, VW1], F16, name="vag", tag="vag")
            nc.sync.dma_start(ktag[:],
                              kagout[:].rearrange("(b p) r -> p b r", p=128))
            nc.sync.dma_start(vag[:],
                              vagout[:].rearrange("(b p) x -> p b x", p=128))

            _sc.__exit__(None, None, None)
            _sc = nc.named_scope(f"attn_{l}"); _sc.__enter__()
            # ---------------- attention ----------------
            attnT = apool.tile([128, FC, R], F16, name="attnT", tag="attnT")
            with (
                tc.tile_pool(name=f"att_{l}", bufs=3) as ap,
                tc.tile_pool(name=f"attd_{l}", bufs=2) as dnp,
                tc.tile_pool(name=f"attps_{l}", bufs=2, space="PSUM") as pst,
                tc.tile_pool(name=f"attpo_{l}", bufs=1, space="PSUM") as pot,
                tc.tile_pool(name=f"attpb_{l}", bufs=2, space="PSUM") as pbb,
            ):
                for g in range(NKV):
                    pos_ = [pot.tile([HD + 1, 2 * R], F32, name=f"po{j}",
                                     tag=f"po{j}") for j in range(GQ // 2)]
                    cb = (g * HD) // 128
                    po_off = (g * HD) % 128
                    for kt in range(NKT):
                        j = (kt * 128) // R
                        lo = kt * 128 - j * R
                        kTc = ktag[po_off:po_off + HD, j * NKB + cb, lo:lo + 128]
                        vpc = vag[:, kt, g * (HD + 1):(g + 1) * (HD + 1)]
                        for hp in range(GQ // 2):
                            hh0 = g * GQ + hp * 2
                            s0 = SLOT[hh0]
                            s1 = SLOT[hh0 + 1]
                            assert s1 == s0 + 2 and (s0 % 2) == (s1 % 2)
                            sT = pst.tile([128, 2 * R], F32, name="sT", tag="sT")
                            p16 = ap.tile([128, 2 * R], F16, name="p16", tag="p16")
                            qpair = qT[(s0 % 2) * 64:(s0 % 2) * 64 + 64,
                                       s0 // 2:s0 // 2 + 2, :]
                            nc.tensor.matmul(sT[:], kTc, qpair,
                                             start=True, stop=True)
                            nc.scalar.activation(p16[:], sT[:],
                                                 mybir.ActivationFunctionType.Exp)
                            nc.vector.tensor_tensor(
                                out=p16[:], in0=p16[:], in1=maskt[:, kt, :],
                                op=mybir.AluOpType.mult)
                            nc.tensor.matmul(
                                pos_[hp][:], vpc, p16[:],
                                start=(kt == 0), stop=(kt == NKT - 1))
                    for h4 in range(GQ):
                        hh = g * GQ + h4
                        po = pos_[h4 // 2]
                        sl = slice((h4 % 2) * R, (h4 % 2 + 1) * R)
                        den = dnp.tile([1, R], F32, name="den", tag="den")
                        nc.vector.tensor_copy(den[:], po[HD:HD + 1, sl])
                        rden = dnp.tile([1, R], F32, name="rden", tag="rden")
                        nc.vector.reciprocal(rden[:], den[:])
                        db = pbb.tile([128, R], F32, name="db", tag="db")
                        nc.tensor.matmul(db[:], one1[:], rden[:],
                                         start=True, stop=True)
                        dbs = dnp.tile([128, R], F32, name="dbs", tag="dbs")
                        nc.scalar.copy(dbs[:], db[:])
                        ss = SLOT[hh]
                        nc.vector.tensor_tensor(
                            out=attnT[(ss % 2) * 64:(ss % 2) * 64 + 64, ss // 2, :],
                            in0=po[0:HD, sl], in1=dbs[0:HD, :],
                            op=mybir.AluOpType.mult)

            _sc.__exit__(None, None, None)
            _sc = nc.named_scope(f"oproj_{l}"); _sc.__enter__()
            # ---------------- o-proj + residual ----------------
            with (
                tc.tile_pool(name=f"ow_{l}", bufs=3) as wpool,
                tc.tile_pool(name=f"ops_{l}", bufs=1, space="PSUM") as pq,
            ):
                for rt in range(NRT):
                    pso = [pq.tile([128, 512], F32, name=f"pso{j}", tag=f"pso{j}")
                           for j in range(NJ)]
                    for fc in range(FC):
                        osl = wpool.tile([128, H], F16, name="osl", tag="osl")
                        nc.sync.dma_start(
                            osl[:], d["ow"][l, fc * 128:(fc + 1) * 128, :])
                        lhsT = attnT[:, fc, rt * 128:(rt + 1) * 128]
                        st = dict(start=(fc == 0), stop=(fc == FC - 1))
                        for j in range(NJ):
                            nc.tensor.matmul(pso[j][:], lhsT,
                                             osl[:, j * 512:(j + 1) * 512], **st)
                    for j in range(NJ):
                        nc.vector.tensor_tensor(
                            out=h[:, rt, j * 512:(j + 1) * 512],
                            in0=pso[j][:], in1=h[:, rt, j * 512:(j + 1) * 512],
                            op=mybir.AluOpType.add)

            _sc.__exit__(None, None, None)
            _sc = nc.named_scope(f"mlp_{l}"); _sc.__enter__()
            # ---------------- mlp ----------------
            normedT = rmsnorm_T(f"l{l}b")
            with (
                tc.tile_pool(name=f"mw_{l}", bufs=3) as wpool,
                tc.tile_pool(name=f"mact_{l}", bufs=3) as mpool,
                tc.tile_pool(name=f"mgu_{l}", bufs=2, space="PSUM") as pgu,
                tc.tile_pool(name=f"mdn_{l}", bufs=1, space="PSUM") as pdn,
            ):
                psd = [pdn.tile([128, 512], F32, name=f"psd{rt}{j}",
                                tag=f"psd{rt}{j}")
                       for rt in range(NRT) for j in range(NJ)]
                for ic in range(IC):
                    if ic % 4 == 0:
                        gsl = wpool.tile([128, FC, 512], F16, name="gsl", tag="gsl")
                        usl = wpool.tile([128, FC, 512], F16, name="usl", tag="usl")
                        nc.sync.dma_start(
                            gsl[:], d["gw"][l].rearrange("(fc p) i -> p fc i", p=128)
                            [:, :, ic * 128:ic * 128 + 512])
                        nc.sync.dma_start(
                            usl[:], d["uw"][l].rearrange("(fc p) i -> p fc i", p=128)
                            [:, :, ic * 128:ic * 128 + 512])
                    io = (ic % 4) * 128
                    psg = pgu.tile([128, R], F32, name="psg", tag="psg")
                    psu = pgu.tile([128, R], F32, name="psu", tag="psu")
                    for fc in range(FC):
                        st = dict(start=(fc == 0), stop=(fc == FC - 1))
                        nc.tensor.matmul(psg[:], gsl[:, fc, io:io + 128],
                                         normedT[:, fc, :], **st)
                        nc.tensor.matmul(psu[:], usl[:, fc, io:io + 128],
                                         normedT[:, fc, :], **st)
                    sg = mpool.tile([128, R], F16, name="sg", tag="sg")
                    nc.scalar.activation(sg[:], psg[:],
                                         mybir.ActivationFunctionType.Sigmoid)
                    gs = mpool.tile([128, R], F16, name="gs", tag="gs")
                    nc.vector.tensor_tensor(out=gs[:], in0=psg[:], in1=sg[:],
                                            op=mybir.AluOpType.mult)
                    actT = mpool.tile([128, R], F16, name="actT", tag="actT")
                    nc.vector.tensor_tensor(out=actT[:], in0=psu[:], in1=gs[:],
                                            op=mybir.AluOpType.mult)
                    dsl = wpool.tile([128, H], F16, name="dsl", tag="dsl")
                    nc.sync.dma_start(dsl[:],
                                      d["dw"][l, ic * 128:(ic + 1) * 128, :])
                    st = dict(start=(ic == 0), stop=(ic == IC - 1))
                    for rt in range(NRT):
                        for j in range(NJ):
                            nc.tensor.matmul(
                                psd[rt * NJ + j][:],
                                actT[:, rt * 128:(rt + 1) * 128],
                                dsl[:, j * 512:(j + 1) * 512], **st)
                for rt in range(NRT):
                    for j in range(NJ):
                        nc.vector.tensor_tensor(
                            out=h[:, rt, j * 512:(j + 1) * 512],
                            in0=psd[rt * NJ + j][:],
                            in1=h[:, rt, j * 512:(j + 1) * 512],
                            op=mybir.AluOpType.add)

            _sc.__exit__(None, None, None)
        # ---------------- final norm + lm head ----------------
        _sc = nc.named_scope("lm"); _sc.__enter__()
        normedT = rmsnorm_T("fin")
        with (
            tc.tile_pool(name="lmw", bufs=3) as wpool,
            tc.tile_pool(name="lmo", bufs=3) as opool,
            tc.tile_pool(name="lmps", bufs=4, space="PSUM") as pl,
        ):
            for vc in range(cfg.NVC):
                w = 512 if vc < cfg.NVC - 1 else cfg.VC_LAST
                lsl = wpool.tile([128, FC, 512], F16, name="lsl", tag="lsl")
                nc.sync.dma_start(
                    lsl[:, :, 0:w],
                    d["lm"].rearrange("(fc p) v -> p fc v", p=128)
                    [:, :, vc * 512:vc * 512 + w])
                for rt in range(NRT):
                    ps = pl.tile([128, 512], F32, name="ps", tag="ps")
                    for fc in range(FC):
                        nc.tensor.matmul(ps[:, 0:w],
                                         normedT[:, fc, rt * 128:(rt + 1) * 128],
                                         lsl[:, fc, 0:w],
                                         start=(fc == 0), stop=(fc == FC - 1))
                    ot = opool.tile([128, 512], F32, name="ot", tag="ot")
                    nc.scalar.copy(ot[:, 0:w], ps[:, 0:w])
                    nc.sync.dma_start(
                        logits[rt * 128:(rt + 1) * 128, vc * 512:vc * 512 + w],
                        ot[:, 0:w])
        _sc.__exit__(None, None, None)


# ---------------------------------------------------------------------------
# host side
# ---------------------------------------------------------------------------

_NC_CACHE = {}


def get_nc(cfg: Cfg):
    key = (cfg.NC, cfg.S, cfg.L, cfg.V, cfg.I)
    if key not in _NC_CACHE:
        _NC_CACHE[key] = build_nc(cfg)
    return _NC_CACHE[key]


def host_prep(inputs, cfg: Cfg):
    inp = {k: np.asarray(v) for k, v in inputs.items()}
    R = cfg.R
    f16 = np.float16

    h0 = inp["embed"][inp["input_ids"][0, :cfg.S]].astype(np.float32).copy()
    off = int(inp["audio_offset"][0])
    h0[off:off + A] = inp["audio_features"][0]

    pos = inp["position_ids"][0, :cfg.S].astype(np.float32)
    inv_freq = 1.0 / (THETA ** (np.arange(0, HD, 2, dtype=np.float32) / HD))
    fr = pos[:, None] * inv_freq[None, :]
    cos32, sin32 = np.cos(fr), np.sin(fr)

    def rope_tables(nw, nheads, scale):
        cos = np.concatenate([cos32, cos32], axis=1)
        sgn_sin = np.concatenate([-sin32, sin32], axis=1)
        partner = np.concatenate([nw[HD // 2:], nw[:HD // 2]])
        ce = cos * (nw * scale)[None, :]
        se = sgn_sin * (partner * scale)[None, :]
        return (np.tile(ce, (1, nheads)).astype(np.float32),
                np.tile(se, (1, nheads)).astype(np.float32))

    cosq = np.stack([rope_tables(inp["qn_w"][l], NH, SCALING)[0]
                     for l in range(cfg.L)])
    sinq = np.stack([rope_tables(inp["qn_w"][l], NH, SCALING)[1]
                     for l in range(cfg.L)])
    cosk = np.stack([rope_tables(inp["kn_w"][l], NKV, 1.0)[0]
                     for l in range(cfg.L)])
    sink = np.stack([rope_tables(inp["kn_w"][l], NKV, 1.0)[1]
                     for l in range(cfg.L)])

    qw = inp["ln1_w"][:cfg.L, :, None] * inp["q_w"][:cfg.L]
    qw = np.ascontiguousarray(
        qw.reshape(cfg.L, H, NH, HD)[:, :, HEAD_AT_SLOT]
        .reshape(cfg.L, H, NH * HD)).astype(f16)
    kw = np.ascontiguousarray(
        inp["ln1_w"][:cfg.L, :, None] * inp["k_w"][:cfg.L]).astype(f16)
    vw = np.ascontiguousarray(
        inp["ln1_w"][:cfg.L, :, None] * inp["v_w"][:cfg.L]).astype(f16)
    ow = np.ascontiguousarray(
        inp["o_w"][:cfg.L].reshape(cfg.L, NH, HD, H)[:, HEAD_AT_SLOT]
        .reshape(cfg.L, NH * HD, H)).astype(f16)
    gw = np.ascontiguousarray(
        inp["ln2_w"][:cfg.L, :, None] * inp["gate_w"][:cfg.L, :, :cfg.I]).astype(f16)
    uw = np.ascontiguousarray(
        inp["ln2_w"][:cfg.L, :, None] * inp["up_w"][:cfg.L, :, :cfg.I]).astype(f16)
    dw = np.ascontiguousarray(inp["down_w"][:cfg.L, :cfg.I]).astype(f16)
    lm = np.ascontiguousarray(
        inp["norm_w"][:, None] * inp["lm_head_w"][:, :cfg.V]).astype(f16)
    ident = np.eye(128, dtype=f16)

    keys = np.arange(cfg.S)
    in_maps = []
    for c in range(cfg.NC):
        rows = np.arange(c * R, (c + 1) * R)
        mask = np.ascontiguousarray(
            (keys[:, None] <= rows[None, :])).astype(f16)
        sl = slice(c * R, (c + 1) * R)
        in_maps.append({
            "h0": h0[sl],
            "cosq": np.ascontiguousarray(cosq[:, sl]),
            "sinq": np.ascontiguousarray(sinq[:, sl]),
            "cosk": np.ascontiguousarray(cosk[:, sl]),
            "sink": np.ascontiguousarray(sink[:, sl]),
            "mask": mask, "ident": ident,
            "qw": qw, "kw": kw, "vw": vw, "ow": ow,
            "gw": gw, "uw": uw, "dw": dw, "lm": lm,
        })
    return in_maps


def assemble(results, cfg: Cfg):
    R = cfg.R
    logits = np.concatenate([results[c]["logits"] for c in range(cfg.NC)],
                            axis=0)[None]  # [B, S, V]
    pk = np.zeros((cfg.L, 1, NKV, cfg.S, HD), np.float32)
    pv = np.zeros((cfg.L, 1, NKV, cfg.S, HD), np.float32)
    for c in range(cfg.NC):
        pk[:, 0, :, c * R:(c + 1) * R] = results[c]["pk"]
        pv[:, 0, :, c * R:(c + 1) * R] = results[c]["pv"]
    return logits, pk, pv


def kernel(**inputs):
    cfg = Cfg()
    nc = get_nc(cfg)
    in_maps = host_prep(inputs, cfg)
    res = bass_utils.run_bass_kernel_spmd(nc, in_maps,
                                          core_ids=list(range(cfg.NC)))
    return assemble(res.results, cfg)


# revision 14
# speedup vs baseline: 1.0305x; 1.0305x over previous
"""Trainium2 Bass kernel for nn_DecoderInitWrapper (4-layer decoder prefill).

Strategy: sequence-parallel across 8 NeuronCores (256 rows each), weights
replicated (fp16), per-layer AllGather of K^T and V' (V with a fused ones
column so the softmax denominator falls out of the PV matmul). Attention is
computed in S^T layout (keys on partitions): softmax needs no max-subtraction
because q/k are RMS-normalized so |s| <= 8; causality is a multiplicative
per-core {0,1} mask applied after exp. All matmuls fp16 with fp32 PSUM
accumulation; residual stream, norms, rope, softmax bookkeeping in fp32.

Host side: embedding gather + audio scatter; RoPE tables with qn/kn and the
1/sqrt(hd) scaling folded in; ln1/ln2/final-norm weights folded into the
following projection matrices.

Self-contained: hardcodes all shapes; reads no files.
"""
import sys

for _p in ("/opt/trn_rl_repo",):
    if _p not in sys.path:
        sys.path.insert(0, _p)

import contextlib

import numpy as np

import concourse.mybir as mybir
import concourse.bacc as bacc
import concourse.tile as tile
from concourse import bass_utils

F32 = mybir.dt.float32
F16 = mybir.dt.float16

B, S, V, H = 1, 2048, 32000, 1024
L, NH, NKV, HD, I = 4, 16, 4, 64, 4096
A = 256
THETA = 10000.0
EPS = 1e-6
SCALING = HD ** -0.5
GQ = NH // NKV
N_CORES = 8

KVW = NKV * HD          # 256

# Head->slot permutation: qT/attnT store head h in slot SLOT[h]; slots with
# even index sit at partition 0, odd at partition 64. Chosen so slot parity
# equals kv-group parity (g = h // GQ), matching kT's partition offset in the
# allgather layout. Host permutes qw columns / ow rows to this order.
HEAD_AT_SLOT = []
for _i in range(NH // 2):
    HEAD_AT_SLOT.append((_i // GQ) * 2 * GQ + (_i % GQ))          # even-g heads
    HEAD_AT_SLOT.append((_i // GQ) * 2 * GQ + GQ + (_i % GQ))     # odd-g heads
SLOT = [0] * NH
for _s, _h in enumerate(HEAD_AT_SLOT):
    SLOT[_h] = _s

VW1 = NKV * (HD + 1)    # 260
AGW = KVW + VW1         # 516


class Cfg:
    def __init__(self, n_cores=N_CORES, s=S, l=L, v=V, i_dim=I):
        self.NC, self.S, self.L, self.V, self.I = n_cores, s, l, v, i_dim
        self.R = s // n_cores
        self.NRT = self.R // 128
        self.NKT = s // 128
        self.FC = H // 128
        self.IC = i_dim // 128
        self.NVC = (v + 511) // 512
        self.VC_LAST = v - 512 * (v // 512) or 512
        assert self.R % 128 == 0


def build_nc(cfg: Cfg):
    nc = bacc.Bacc("TRN2", target_bir_lowering=False, debug=False,
                   num_devices=cfg.NC)
    d = {}
    R, Ldim, Vdim, Idim = cfg.R, cfg.L, cfg.V, cfg.I

    def inp(name, shape, dt=F16):
        d[name] = nc.dram_tensor(name, shape, dt, kind="ExternalInput").ap()

    inp("h0", [R, H], F32)
    inp("cosq", [Ldim, R, NH * HD], F32)
    inp("sinq", [Ldim, R, NH * HD], F32)
    inp("cosk", [Ldim, R, KVW], F32)
    inp("sink", [Ldim, R, KVW], F32)
    inp("mask", [cfg.S, R], F16)
    inp("ident", [128, 128], F16)
    inp("qw", [Ldim, H, NH * HD])
    inp("kw", [Ldim, H, KVW])
    inp("vw", [Ldim, H, KVW])
    inp("ow", [Ldim, NH * HD, H])
    inp("gw", [Ldim, H, Idim])
    inp("uw", [Ldim, H, Idim])
    inp("dw", [Ldim, Idim, H])
    inp("lm", [H, Vdim])

    logits = nc.dram_tensor("logits", [R, Vdim], F32, kind="ExternalOutput").ap()
    pk = nc.dram_tensor("pk", [Ldim, NKV, R, HD], F32, kind="ExternalOutput").ap()
    pv = nc.dram_tensor("pv", [Ldim, NKV, R, HD], F32, kind="ExternalOutput").ap()

    with tile.TileContext(nc) as tc:
        _emit(nc, tc, cfg, d, logits, pk, pv)
    nc.compile()
    return nc


def _emit(nc, tc, cfg, d, logits, pk, pv):
    R, NRT, NKT, FC, IC = cfg.R, cfg.NRT, cfg.NKT, cfg.FC, cfg.IC
    Ldim = cfg.L
    NJ = H // 512  # 512-chunks of H

    ctx = contextlib.ExitStack()
    with ctx:
        cpool = ctx.enter_context(tc.tile_pool(name="const", bufs=1))
        ident = cpool.tile([128, 128], F16, name="ident")
        nc.sync.dma_start(ident[:], d["ident"][:])
        one1 = cpool.tile([1, 128], F32, name="one1")
        nc.gpsimd.memset(one1[:], 1.0)
        epst = cpool.tile([128, 1], F32, name="epst")
        nc.gpsimd.memset(epst[:], EPS)
        maskt = cpool.tile([128, NKT, 2 * R], F16, name="maskt")
        nc.sync.dma_start(maskt[:, :, 0:R],
                          d["mask"].rearrange("(kt p) r -> p kt r", p=128))
        nc.sync.dma_start(maskt[:, :, R:2 * R],
                          d["mask"].rearrange("(kt p) r -> p kt r", p=128))
        h = cpool.tile([128, NRT, H], F32, name="h")
        nc.sync.dma_start(h[:], d["h0"].rearrange("(rt p) x -> p rt x", p=128))

        apool = ctx.enter_context(tc.tile_pool(name="acts", bufs=1))
        rpool = ctx.enter_context(tc.tile_pool(name="ropetabs", bufs=2))
        dpool = ctx.enter_context(tc.tile_pool(name="dramp", bufs=2, space="DRAM"))

        def rmsnorm_T(tag):
            """normedT [128, FC, R] f16 = transpose(h * rsqrt(mean(h^2)+eps))."""
            normedT = apool.tile([128, FC, R], F16, name="normedT", tag="normedT")
            with (
                tc.tile_pool(name=f"rms_{tag}", bufs=2) as rp,
                tc.tile_pool(name=f"rmsps_{tag}", bufs=2, space="PSUM") as pp,
            ):
                for rt in range(NRT):
                    sq = rp.tile([128, H], F16, name="sq", tag="sq")
                    var = rp.tile([128, 1], F32, name="var", tag="var")
                    nc.scalar.activation(sq[:], h[:, rt, :],
                                         mybir.ActivationFunctionType.Square,
                                         scale=float(1.0 / np.sqrt(H)),
                                         accum_out=var[:])
                    std = rp.tile([128, 1], F32, name="std", tag="std")
                    nc.scalar.activation(std[:], var[:],
                                         mybir.ActivationFunctionType.Sqrt,
                                         bias=epst[:])
                    rs = rp.tile([128, 1], F32, name="rs", tag="rs")
                    nc.vector.reciprocal(rs[:], std[:])
                    normed = rp.tile([128, H], F16, name="normed", tag="normed")
                    nc.vector.tensor_scalar_mul(normed[:], h[:, rt, :], rs[:])
                    for fc in range(FC):
                        tp = pp.tile([128, 128], F16, name="tp", tag="tp")
                        nc.tensor.transpose(tp[:], normed[:, fc * 128:(fc + 1) * 128],
                                            ident[:])
                        nc.vector.tensor_copy(
                            normedT[:, fc, rt * 128:(rt + 1) * 128], tp[:])
            return normedT

        def rope_inplace(dst_ap, xn, cos_ap, sin_ap, nheads, f32_copy=None):
            """dst = xn*cos + rotview(xn)*sin; all [128, nheads*64] fp32 views."""
            with tc.tile_pool(name="ropetmp", bufs=2) as rp:
                W = nheads * HD
                t1 = rp.tile([128, W], F32, name="t1", tag=f"t1{nheads}")
                nc.vector.tensor_tensor(out=t1[:], in0=xn[:], in1=cos_ap,
                                        op=mybir.AluOpType.mult)
                t2 = rp.tile([128, W], F32, name="t2", tag=f"t2{nheads}")
                xv = xn[:].rearrange("p (h b x) -> p h b x", h=nheads, b=2, x=32)
                sv = sin_ap.rearrange("p (h b x) -> p h b x", h=nheads, b=2, x=32)
                tv = t2[:].rearrange("p (h b x) -> p h b x", h=nheads, b=2, x=32)
                # block-swapped multiply (rotate-half with sign folded into sin)
                nc.vector.tensor_tensor(out=tv[:, :, 0, :], in0=xv[:, :, 1, :],
                                        in1=sv[:, :, 0, :], op=mybir.AluOpType.mult)
                nc.vector.tensor_tensor(out=tv[:, :, 1, :], in0=xv[:, :, 0, :],
                                        in1=sv[:, :, 1, :], op=mybir.AluOpType.mult)
                nc.vector.tensor_tensor(out=dst_ap, in0=t1[:], in1=t2[:],
                                        op=mybir.AluOpType.add)
                if f32_copy is not None:
                    nc.vector.tensor_tensor(out=f32_copy, in0=t1[:], in1=t2[:],
                                            op=mybir.AluOpType.add)

        for l in range(Ldim):
            # rope tables for this layer
            cosq = rpool.tile([128, NRT, NH * HD], F32, name="cosq", tag="cosq")
            sinq = rpool.tile([128, NRT, NH * HD], F32, name="sinq", tag="sinq")
            cosk = rpool.tile([128, NRT, KVW], F32, name="cosk", tag="cosk")
            sink = rpool.tile([128, NRT, KVW], F32, name="sink", tag="sink")
            for nm, t in (("cosq", cosq), ("sinq", sinq),
                          ("cosk", cosk), ("sink", sink)):
                nc.sync.dma_start(t[:],
                                  d[nm][l].rearrange("(rt p) x -> p rt x", p=128))

            # ---------------- ln1 + qkv + rope ----------------
            _sc1 = nc.named_scope(f"ln1_{l}"); _sc1.__enter__()
            normedT = rmsnorm_T(f"l{l}a")
            _sc1.__exit__(None, None, None)
            _sc = nc.named_scope(f"qkv_{l}"); _sc.__enter__()
            q16 = apool.tile([128, NRT, NH * HD], F16, name="q16", tag="q16")
            k16 = apool.tile([128, NRT, KVW], F16, name="k16", tag="k16")
            kf32 = apool.tile([128, NRT, KVW], F32, name="kf32", tag="kf32")
            vf = apool.tile([128, NRT, VW1], F16, name="vf", tag="vf")
            nc.gpsimd.memset(
                vf[:].rearrange("p rt (g x) -> p rt g x", g=NKV, x=HD + 1)
                [:, :, :, HD:HD + 1], 1.0)

            with (
                tc.tile_pool(name=f"qkvw_{l}", bufs=3) as wpool,
                tc.tile_pool(name=f"qkvt_{l}", bufs=2) as tpool,
                tc.tile_pool(name=f"qkvp_{l}", bufs=2, space="PSUM") as pq,
            ):
                for rt in range(NRT):
                    psq = [pq.tile([128, 512], F32, name=f"psq{j}", tag=f"psq{j}")
                           for j in range(NH * HD // 512)]
                    psk = pq.tile([128, KVW], F32, name="psk", tag="psk")
                    psv = pq.tile([128, KVW], F32, name="psv", tag="psv")
                    for fc in range(FC):
                        wsl = wpool.tile([128, NH * HD + 2 * KVW], F16,
                                         name="wsl", tag="wsl")
                        nc.sync.dma_start(wsl[:, 0:NH * HD],
                                          d["qw"][l, fc * 128:(fc + 1) * 128, :])
                        nc.sync.dma_start(wsl[:, NH * HD:NH * HD + KVW],
                                          d["kw"][l, fc * 128:(fc + 1) * 128, :])
                        nc.sync.dma_start(wsl[:, NH * HD + KVW:],
                                          d["vw"][l, fc * 128:(fc + 1) * 128, :])
                        lhsT = normedT[:, fc, rt * 128:(rt + 1) * 128]
                        st = dict(start=(fc == 0), stop=(fc == FC - 1))
                        for j in range(NH * HD // 512):
                            nc.tensor.matmul(psq[j][:], lhsT,
                                             wsl[:, j * 512:(j + 1) * 512], **st)
                        nc.tensor.matmul(psk[:], lhsT,
                                         wsl[:, NH * HD:NH * HD + KVW], **st)
                        nc.tensor.matmul(psv[:], lhsT,
                                         wsl[:, NH * HD + KVW:], **st)

                    # ---- q epilogue: per-head rms + rope -> q16
                    xnq = tpool.tile([128, NH * HD], F32, name="xnq", tag="xnq")
                    ssq = tpool.tile([128, NH], F32, name="ssq", tag="ssq")
                    scr = tpool.tile([128, HD], F16, name="scr", tag="scr")
                    for hh in range(NH):
                        bank = psq[(hh * HD) // 512]
                        off = (hh * HD) % 512
                        nc.scalar.activation(scr[:], bank[:, off:off + HD],
                                             mybir.ActivationFunctionType.Square,
                                             scale=float(1.0 / np.sqrt(HD)),
                                             accum_out=ssq[:, hh:hh + 1])
                    stq = tpool.tile([128, NH], F32, name="stq", tag="stq")
                    nc.scalar.activation(stq[:], ssq[:],
                                         mybir.ActivationFunctionType.Sqrt,
                                         bias=epst[:])
                    rsq = tpool.tile([128, NH], F32, name="rsq", tag="rsq")
                    nc.vector.reciprocal(rsq[:], stq[:])
                    for hh in range(NH):
                        bank = psq[(hh * HD) // 512]
                        off = (hh * HD) % 512
                        nc.vector.tensor_scalar_mul(
                            xnq[:, hh * HD:(hh + 1) * HD],
                            bank[:, off:off + HD], rsq[:, hh:hh + 1])
                    rope_inplace(q16[:, rt, :], xnq, cosq[:, rt, :],
                                 sinq[:, rt, :], NH)

                    # ---- k epilogue
                    xnk = tpool.tile([128, KVW], F32, name="xnk", tag="xnk")
                    ssk = tpool.tile([128, NKV], F32, name="ssk", tag="ssk")
                    for hh in range(NKV):
                        nc.scalar.activation(scr[:], psk[:, hh * HD:(hh + 1) * HD],
                                             mybir.ActivationFunctionType.Square,
                                             scale=float(1.0 / np.sqrt(HD)),
                                             accum_out=ssk[:, hh:hh + 1])
                    stk = tpool.tile([128, NKV], F32, name="stk", tag="stk")
                    nc.scalar.activation(stk[:], ssk[:],
                                         mybir.ActivationFunctionType.Sqrt,
                                         bias=epst[:])
                    rsk = tpool.tile([128, NKV], F32, name="rsk", tag="rsk")
                    nc.vector.reciprocal(rsk[:], stk[:])
                    for hh in range(NKV):
                        nc.vector.tensor_scalar_mul(
                            xnk[:, hh * HD:(hh + 1) * HD],
                            psk[:, hh * HD:(hh + 1) * HD], rsk[:, hh:hh + 1])
                    rope_inplace(k16[:, rt, :], xnk, cosk[:, rt, :],
                                 sink[:, rt, :], NKV, f32_copy=kf32[:, rt, :])

                    # ---- v epilogue
                    vf32 = tpool.tile([128, KVW], F32, name="vf32", tag="vf32")
                    nc.scalar.copy(vf32[:], psv[:])
                    nc.sync.dma_start(
                        pv[l].rearrange("g (rt p) x -> p rt g x", p=128)[:, rt],
                        vf32[:].rearrange("p (g x) -> p g x", g=NKV))
                    nc.vector.tensor_copy(
                        vf[:, rt, :].rearrange("p (g x) -> p g x", g=NKV, x=HD + 1)
                        [:, :, 0:HD],
                        psv[:].rearrange("p (g x) -> p g x", g=NKV))
                for rt in range(NRT):
                    nc.sync.dma_start(
                        pk[l].rearrange("g (rt p) x -> p rt g x", p=128)[:, rt],
                        kf32[:, rt, :].rearrange("p (g x) -> p g x", g=NKV))

            _sc.__exit__(None, None, None)
            _sc = nc.named_scope(f"tps_{l}"); _sc.__enter__()
            # ---------------- transposes: qT, kT ----------------
            qT = apool.tile([128, FC, R], F16, name="qT", tag="qT")
            kT = apool.tile([128, KVW // 128, R], F16, name="kT", tag="kT")
            with tc.tile_pool(name=f"tps_{l}", bufs=2, space="PSUM") as pp:
                for rt in range(NRT):
                    for fc in range(FC):
                        tp = pp.tile([128, 128], F16, name="tp", tag="tp")
                        nc.tensor.transpose(
                            tp[:], q16[:, rt, fc * 128:(fc + 1) * 128], ident[:])
                        nc.vector.tensor_copy(
                            qT[:, fc, rt * 128:(rt + 1) * 128], tp[:])
                    for cb in range(KVW // 128):
                        tp = pp.tile([128, 128], F16, name="tp", tag="tp")
                        nc.tensor.transpose(
                            tp[:], k16[:, rt, cb * 128:(cb + 1) * 128], ident[:])
                        nc.vector.tensor_copy(
                            kT[:, cb, rt * 128:(rt + 1) * 128], tp[:])

            _sc.__exit__(None, None, None)
            _sc = nc.named_scope(f"ag_{l}"); _sc.__enter__()
            # ---------------- AllGather kT and v' ----------------
            shared = "Shared" if cfg.NC > 4 else "Local"
            kagin = dpool.tile([KVW, R], F16, name="kagin", tag="kagin")
            vagin = dpool.tile([R, VW1], F16, name="vagin", tag="vagin")
            kagout = dpool.tile([cfg.NC * KVW, R], F16, name="kagout",
                                tag="kagout", addr_space=shared)
            vagout = dpool.tile([cfg.NC * R, VW1], F16, name="vagout",
                                tag="vagout", addr_space=shared)
            nc.sync.dma_start(
                kagin[:].rearrange("(cb p) r -> p cb r", p=128), kT[:])
            nc.sync.dma_start(
                vagin[:].rearrange("(rt p) x -> p rt x", p=128), vf[:])
            nc.gpsimd.collective_compute(
                "AllGather", mybir.AluOpType.bypass,
                replica_groups=[list(range(cfg.NC))],
                ins=[kagin.opt()], outs=[kagout.opt()])
            nc.gpsimd.collective_compute(
                "AllGather", mybir.AluOpType.bypass,
                replica_groups=[list(range(cfg.NC))],
                ins=[vagin.opt()], outs=[vagout.opt()])
            NKB = KVW // 128  # kT partition blocks per core
            ktag = apool.tile([128, cfg.NC * NKB, R], F16, name="ktag",
                              tag="ktag")
            vag = apool.tile([128, NKT, VW1], F16, name="vag", tag="vag")
            nc.sync.dma_start(ktag[:],
                              kagout[:].rearrange("(b p) r -> p b r", p=128))
            nc.sync.dma_start(vag[:],
                              vagout[:].rearrange("(b p) x -> p b x", p=128))

            _sc.__exit__(None, None, None)
            _sc = nc.named_scope(f"attn_{l}"); _sc.__enter__()
            # ---------------- attention ----------------
            attnT = apool.tile([128, FC, R], F16, name="attnT", tag="attnT")
            with (
                tc.tile_pool(name=f"att_{l}", bufs=3) as ap,
                tc.tile_pool(name=f"attd_{l}", bufs=2) as dnp,
                tc.tile_pool(name=f"attps_{l}", bufs=2, space="PSUM") as pst,
                tc.tile_pool(name=f"attpo_{l}", bufs=1, space="PSUM") as pot,
                tc.tile_pool(name=f"attpb_{l}", bufs=2, space="PSUM") as pbb,
            ):
                for g in range(NKV):
                    pos_ = [pot.tile([HD + 1, R], F32, name=f"po{j}",
                                     tag=f"po{j}") for j in range(GQ)]
                    cb = (g * HD) // 128
                    po_off = (g * HD) % 128
                    for kt in range(NKT):
                        j = (kt * 128) // R
                        lo = kt * 128 - j * R
                        kTc = ktag[po_off:po_off + HD, j * NKB + cb, lo:lo + 128]
                        vpc = vag[:, kt, g * (HD + 1):(g + 1) * (HD + 1)]
                        for hp in range(GQ // 2):
                            sT = pst.tile([128, 2 * R], F32, name="sT", tag="sT")
                            p16 = ap.tile([128, 2 * R], F16, name="p16", tag="p16")
                            for hx in range(2):
                                hh = g * GQ + hp * 2 + hx
                                ss = SLOT[hh]
                                nc.tensor.matmul(
                                    sT[:, hx * R:(hx + 1) * R], kTc,
                                    qT[(ss % 2) * 64:(ss % 2) * 64 + 64, ss // 2, :],
                                    start=True, stop=True)
                            nc.scalar.activation(p16[:], sT[:],
                                                 mybir.ActivationFunctionType.Exp)
                            nc.vector.tensor_tensor(
                                out=p16[:], in0=p16[:], in1=maskt[:, kt, :],
                                op=mybir.AluOpType.mult)
                            for hx in range(2):
                                nc.tensor.matmul(
                                    pos_[hp * 2 + hx][:], vpc,
                                    p16[:, hx * R:(hx + 1) * R],
                                    start=(kt == 0), stop=(kt == NKT - 1))
                    for h4 in range(GQ):
                        hh = g * GQ + h4
                        po = pos_[h4]
                        sl = slice(0, R)
                        den = dnp.tile([1, R], F32, name="den", tag="den")
                        nc.vector.tensor_copy(den[:], po[HD:HD + 1, sl])
                        rden = dnp.tile([1, R], F32, name="rden", tag="rden")
                        nc.vector.reciprocal(rden[:], den[:])
                        db = pbb.tile([128, R], F32, name="db", tag="db")
                        nc.tensor.matmul(db[:], one1[:], rden[:],
                                         start=True, stop=True)
                        dbs = dnp.tile([128, R], F32, name="dbs", tag="dbs")
                        nc.scalar.copy(dbs[:], db[:])
                        ss = SLOT[hh]
                        nc.vector.tensor_tensor(
                            out=attnT[(ss % 2) * 64:(ss % 2) * 64 + 64, ss // 2, :],
                            in0=po[0:HD, sl], in1=dbs[0:HD, :],
                            op=mybir.AluOpType.mult)

            _sc.__exit__(None, None, None)
            _sc = nc.named_scope(f"oproj_{l}"); _sc.__enter__()
            # ---------------- o-proj + residual ----------------
            with (
                tc.tile_pool(name=f"ow_{l}", bufs=3) as wpool,
                tc.tile_pool(name=f"ops_{l}", bufs=1, space="PSUM") as pq,
            ):
                for rt in range(NRT):
                    pso = [pq.tile([128, 512], F32, name=f"pso{j}", tag=f"pso{j}")
                           for j in range(NJ)]
                    for fc in range(FC):
                        osl = wpool.tile([128, H], F16, name="osl", tag="osl")
                        nc.sync.dma_start(
                            osl[:], d["ow"][l, fc * 128:(fc + 1) * 128, :])
                        lhsT = attnT[:, fc, rt * 128:(rt + 1) * 128]
                        st = dict(start=(fc == 0), stop=(fc == FC - 1))
                        for j in range(NJ):
                            nc.tensor.matmul(pso[j][:], lhsT,
                                             osl[:, j * 512:(j + 1) * 512], **st)
                    for j in range(NJ):
                        nc.vector.tensor_tensor(
                            out=h[:, rt, j * 512:(j + 1) * 512],
                            in0=pso[j][:], in1=h[:, rt, j * 512:(j + 1) * 512],
                            op=mybir.AluOpType.add)

            _sc.__exit__(None, None, None)
            _sc = nc.named_scope(f"mlp_{l}"); _sc.__enter__()
            # ---------------- mlp ----------------
            normedT = rmsnorm_T(f"l{l}b")
            with (
                tc.tile_pool(name=f"mw_{l}", bufs=3) as wpool,
                tc.tile_pool(name=f"mact_{l}", bufs=3) as mpool,
                tc.tile_pool(name=f"mgu_{l}", bufs=2, space="PSUM") as pgu,
                tc.tile_pool(name=f"mdn_{l}", bufs=1, space="PSUM") as pdn,
            ):
                psd = [pdn.tile([128, 512], F32, name=f"psd{rt}{j}",
                                tag=f"psd{rt}{j}")
                       for rt in range(NRT) for j in range(NJ)]
                for ic in range(IC):
                    if ic % 4 == 0:
                        gsl = wpool.tile([128, FC, 512], F16, name="gsl", tag="gsl")
                        usl = wpool.tile([128, FC, 512], F16, name="usl", tag="usl")
                        nc.sync.dma_start(
                            gsl[:], d["gw"][l].rearrange("(fc p) i -> p fc i", p=128)
                            [:, :, ic * 128:ic * 128 + 512])
                        nc.sync.dma_start(
                            usl[:], d["uw"][l].rearrange("(fc p) i -> p fc i", p=128)
                            [:, :, ic * 128:ic * 128 + 512])
                    io = (ic % 4) * 128
                    psg = pgu.tile([128, R], F32, name="psg", tag="psg")
                    psu = pgu.tile([128, R], F32, name="psu", tag="psu")
                    for fc in range(FC):
                        st = dict(start=(fc == 0), stop=(fc == FC - 1))
                        nc.tensor.matmul(psg[:], gsl[:, fc, io:io + 128],
                                         normedT[:, fc, :], **st)
                        nc.tensor.matmul(psu[:], usl[:, fc, io:io + 128],
                                         normedT[:, fc, :], **st)
                    sg = mpool.tile([128, R], F16, name="sg", tag="sg")
                    nc.scalar.activation(sg[:], psg[:],
                                         mybir.ActivationFunctionType.Sigmoid)
                    gs = mpool.tile([128, R], F16, name="gs", tag="gs")
                    nc.vector.tensor_tensor(out=gs[:], in0=psg[:], in1=sg[:],
                                            op=mybir.AluOpType.mult)
                    actT = mpool.tile([128, R], F16, name="actT", tag="actT")
                    nc.vector.tensor_tensor(out=actT[:], in0=psu[:], in1=gs[:],
                                            op=mybir.AluOpType.mult)
                    dsl = wpool.tile([128, H], F16, name="dsl", tag="dsl")
                    nc.sync.dma_start(dsl[:],
                                      d["dw"][l, ic * 128:(ic + 1) * 128, :])
                    st = dict(start=(ic == 0), stop=(ic == IC - 1))
                    for rt in range(NRT):
                        for j in range(NJ):
                            nc.tensor.matmul(
                                psd[rt * NJ + j][:],
                                actT[:, rt * 128:(rt + 1) * 128],
                                dsl[:, j * 512:(j + 1) * 512], **st)
                for rt in range(NRT):
                    for j in range(NJ):
                        nc.vector.tensor_tensor(
                            out=h[:, rt, j * 512:(j + 1) * 512],
                            in0=psd[rt * NJ + j][:],
                            in1=h[:, rt, j * 512:(j + 1) * 512],
                            op=mybir.AluOpType.add)

            _sc.__exit__(None, None, None)
        # ---------------- final norm + lm head ----------------
        _sc = nc.named_scope("lm"); _sc.__enter__()
        normedT = rmsnorm_T("fin")
        with (
            tc.tile_pool(name="lmw", bufs=3) as wpool,
            tc.tile_pool(name="lmo", bufs=3) as opool,
            tc.tile_pool(name="lmps", bufs=4, space="PSUM") as pl,
        ):
            for vc in range(cfg.NVC):
                w = 512 if vc < cfg.NVC - 1 else cfg.VC_LAST
                lsl = wpool.tile([128, FC, 512], F16, name="lsl", tag="lsl")
                nc.sync.dma_start(
                    lsl[:, :, 0:w],
                    d["lm"].rearrange("(fc p) v -> p fc v", p=128)
                    [:, :, vc * 512:vc * 512 + w])
                for rt in range(NRT):
                    ps = pl.tile([128, 512], F32, name="ps", tag="ps")
                    for fc in range(FC):
                        nc.tensor.matmul(ps[:, 0:w],
                                         normedT[:, fc, rt * 128:(rt + 1) * 128],
                                         lsl[:, fc, 0:w],
                                         start=(fc == 0), stop=(fc == FC - 1))
                    ot = opool.tile([128, 512], F32, name="ot", tag="ot")
                    nc.scalar.copy(ot[:, 0:w], ps[:, 0:w])
                    nc.sync.dma_start(
                        logits[rt * 128:(rt + 1) * 128, vc * 512:vc * 512 + w],
                        ot[:, 0:w])
        _sc.__exit__(None, None, None)


# ---------------------------------------------------------------------------
# host side
# ---------------------------------------------------------------------------

_NC_CACHE = {}


def get_nc(cfg: Cfg):
    key = (cfg.NC, cfg.S, cfg.L, cfg.V, cfg.I)
    if key not in _NC_CACHE:
        _NC_CACHE[key] = build_nc(cfg)
    return _NC_CACHE[key]


def host_prep(inputs, cfg: Cfg):
    inp = {k: np.asarray(v) for k, v in inputs.items()}
    R = cfg.R
    f16 = np.float16

    h0 = inp["embed"][inp["input_ids"][0, :cfg.S]].astype(np.float32).copy()
    off = int(inp["audio_offset"][0])
    h0[off:off + A] = inp["audio_features"][0]

    pos = inp["position_ids"][0, :cfg.S].astype(np.float32)
    inv_freq = 1.0 / (THETA ** (np.arange(0, HD, 2, dtype=np.float32) / HD))
    fr = pos[:, None] * inv_freq[None, :]
    cos32, sin32 = np.cos(fr), np.sin(fr)

    def rope_tables(nw, nheads, scale):
        cos = np.concatenate([cos32, cos32], axis=1)
        sgn_sin = np.concatenate([-sin32, sin32], axis=1)
        partner = np.concatenate([nw[HD // 2:], nw[:HD // 2]])
        ce = cos * (nw * scale)[None, :]
        se = sgn_sin * (partner * scale)[None, :]
        return (np.tile(ce, (1, nheads)).astype(np.float32),
                np.tile(se, (1, nheads)).astype(np.float32))

    cosq = np.stack([rope_tables(inp["qn_w"][l], NH, SCALING)[0]
                     for l in range(cfg.L)])
    sinq = np.stack([rope_tables(inp["qn_w"][l], NH, SCALING)[1]
                     for l in range(cfg.L)])
    cosk = np.stack([rope_tables(inp["kn_w"][l], NKV, 1.0)[0]
                     for l in range(cfg.L)])
    sink = np.stack([rope_tables(inp["kn_w"][l], NKV, 1.0)[1]
                     for l in range(cfg.L)])

    qw = inp["ln1_w"][:cfg.L, :, None] * inp["q_w"][:cfg.L]
    qw = np.ascontiguousarray(
        qw.reshape(cfg.L, H, NH, HD)[:, :, HEAD_AT_SLOT]
        .reshape(cfg.L, H, NH * HD)).astype(f16)
    kw = np.ascontiguousarray(
        inp["ln1_w"][:cfg.L, :, None] * inp["k_w"][:cfg.L]).astype(f16)
    vw = np.ascontiguousarray(
        inp["ln1_w"][:cfg.L, :, None] * inp["v_w"][:cfg.L]).astype(f16)
    ow = np.ascontiguousarray(
        inp["o_w"][:cfg.L].reshape(cfg.L, NH, HD, H)[:, HEAD_AT_SLOT]
        .reshape(cfg.L, NH * HD, H)).astype(f16)
    gw = np.ascontiguousarray(
        inp["ln2_w"][:cfg.L, :, None] * inp["gate_w"][:cfg.L, :, :cfg.I]).astype(f16)
    uw = np.ascontiguousarray(
        inp["ln2_w"][:cfg.L, :, None] * inp["up_w"][:cfg.L, :, :cfg.I]).astype(f16)
    dw = np.ascontiguousarray(inp["down_w"][:cfg.L, :cfg.I]).astype(f16)
    lm = np.ascontiguousarray(
        inp["norm_w"][:, None] * inp["lm_head_w"][:, :cfg.V]).astype(f16)
    ident = np.eye(128, dtype=f16)

    keys = np.arange(cfg.S)
    in_maps = []
    for c in range(cfg.NC):
        rows = np.arange(c * R, (c + 1) * R)
        mask = np.ascontiguousarray(
            (keys[:, None] <= rows[None, :])).astype(f16)
        sl = slice(c * R, (c + 1) * R)
        in_maps.append({
            "h0": h0[sl],
            "cosq": np.ascontiguousarray(cosq[:, sl]),
            "sinq": np.ascontiguousarray(sinq[:, sl]),
            "cosk": np.ascontiguousarray(cosk[:, sl]),
            "sink": np.ascontiguousarray(sink[:, sl]),
            "mask": mask, "ident": ident,
            "qw": qw, "kw": kw, "vw": vw, "ow": ow,
            "gw": gw, "uw": uw, "dw": dw, "lm": lm,
        })
    return in_maps


def assemble(results, cfg: Cfg):
    R = cfg.R
    logits = np.concatenate([results[c]["logits"] for c in range(cfg.NC)],
                            axis=0)[None]  # [B, S, V]
    pk = np.zeros((cfg.L, 1, NKV, cfg.S, HD), np.float32)
    pv = np.zeros((cfg.L, 1, NKV, cfg.S, HD), np.float32)
    for c in range(cfg.NC):
        pk[:, 0, :, c * R:(c + 1) * R] = results[c]["pk"]
        pv[:, 0, :, c * R:(c + 1) * R] = results[c]["pv"]
    return logits, pk, pv


def kernel(**inputs):
    cfg = Cfg()
    nc = get_nc(cfg)
    in_maps = host_prep(inputs, cfg)
    res = bass_utils.run_bass_kernel_spmd(nc, in_maps,
                                          core_ids=list(range(cfg.NC)))
    return assemble(res.results, cfg)
